# revision 7
# baseline (speedup 1.0000x reference)
import sys

for p in ("/opt/trn_rl_repo",):
    if p not in sys.path:
        sys.path.insert(0, p)

import math
from contextlib import ExitStack

import numpy as np

import concourse.bass as bass
import concourse.bacc as bacc
import concourse.mybir as mybir
from concourse.bass_utils import run_bass_kernel_spmd

# ---------------- problem constants ----------------
N, E, IN, H, C, L = 40000, 640000, 256, 128, 64, 4
ALPHA, LAMB = 0.5, 1.0

NC = 8
PER = 5000            # real nodes per core
PERP = 5120           # padded nodes per core
TILES = PERP // 128   # 40
NTAB = NC * PERP      # 40960 global table rows
REG = 2
REG_ROWS = NTAB // REG  # 20480 rows per gather region (int16-safe)

D = 8                 # slots per bin
RANK_CAPS = [5120, 2560, 128, 128, 128]   # bins per rank per region (128-aligned)
CAPB = np.concatenate([[0], np.cumsum(RANK_CAPS)]).astype(np.int64)  # rank bases
NBINS_REG = 8192      # region bin block (spare bins above sum(RANK_CAPS))
NBINS = REG * NBINS_REG          # 16384
SLOTS_CALL = 8192                # gather slots per call
BINS_CALL = SLOTS_CALL // D      # 1024
CALLS = NBINS * D // SLOTS_CALL  # 16
CALLS_REG = CALLS // REG         # 8
MM_PER_CALL = 16                 # each matmul: 4 cols x 128 parts = 512 slots = 64 bins
DRAINS_PER_CALL = 2              # each drain: 8 matmuls = 512 bins
ST2_COLS = NBINS // 128          # 128

ZROW = 8 * TILES + 39            # rel table row of node 5000 (zeroed) in shard 0 of a region
TRASH = (5100 % 128) * TILES + 5100 // 128  # agg row of trash node 5100

# scatter blocks: (region, rank_q[0-based], label); A-rank0 is the r1 plain DMA
SCAT_BLOCKS = [
    (1, 0), (0, 1), (1, 1), (0, 2), (1, 2), (0, 3), (1, 3), (0, 4), (1, 4),
]
SCAT_VBASE = [R * NBINS_REG + int(CAPB[q]) for (R, q) in SCAT_BLOCKS]
SCAT_CAP = [RANK_CAPS[q] for (_, q) in SCAT_BLOCKS]
SCAT_COL0 = [vb // 128 for vb in SCAT_VBASE]
SCAT_IDX_OFF = np.concatenate([[0], np.cumsum([c // 16 for c in SCAT_CAP])]).astype(int)
SCAT_IDX_COLS = int(SCAT_IDX_OFF[-1])  # total wrapped idx cols (688)

F32 = mybir.dt.float32
BF16 = mybir.dt.bfloat16
I16 = mybir.dt.int16

_BF16NP = mybir.dt.np(BF16)


def _vc_to_slot0(vc):
    """in-call bin index vc (0..1023) -> first slot index of its 8-column bin.

    Bin (p=vc%128, c=vc//128) occupies slots (8c+i)*128 + p for i in 0..7;
    the per-call DVE tensor_reduce sums i and writes [p, c] of the call's
    8-column stage2 block, so global v = 1024*call + c*128 + p matches the
    stage2 slot layout directly.
    """
    p = vc % 128
    c = vc // 128
    return (8 * c) * 128 + p


def _wrap16(a, width):
    """1-D int array (len multiple of 16) -> [128, len/16] wrapped+replicated."""
    a = np.asarray(a, np.int16)
    blk = a.reshape(-1, 16).T
    out = np.tile(blk, (8, 1))
    assert out.shape == (128, width)
    return np.ascontiguousarray(out)


def _node_row(n):
    """local node id -> (p*TILES + t) row index used for table/agg layouts."""
    return (n % 128) * TILES + n // 128


def _prep_core(dloc, rel, region):
    """Per-core edge preprocessing.

    dloc:   [e] local dst node (0..PER-1)
    rel:    [e] region-relative table row of the source
    region: [e] gather region (0/1)

    Returns (slots[CALLS,SLOTS_CALL] int16, sidx[128, SCAT_IDX_COLS] int16).
    """
    slots = np.full((CALLS, SLOTS_CALL), ZROW, np.int16)
    sidx_blocks = []
    for R in range(REG):
        m = region == R
        dl = dloc[m]
        rr = rel[m]
        order = np.argsort(dl, kind="stable")
        dl = dl[order]
        rr = rr[order]
        cnt = np.bincount(dl, minlength=PERP)
        nb = (cnt + D - 1) // D
        if R == 0:
            nb = np.maximum(nb, 1)  # every dst owns a rank-0 bin in region A
        # per-rank positions
        nq = len(RANK_CAPS)
        posmat = np.zeros((nq, PERP), np.int64)
        dstlists = []
        for q in range(nq):
            mask = nb > q
            cnt_q = int(mask.sum())
            if cnt_q > RANK_CAPS[q]:
                raise RuntimeError(f"rank cap overflow: region {R} rank {q}: {cnt_q}")
            posmat[q] = np.cumsum(mask) - 1
            dstlists.append(np.where(mask)[0])
        if np.any(nb > nq):
            raise RuntimeError("bin rank overflow (degree too high)")
        # per-edge bin placement
        starts = np.zeros(PERP + 1, np.int64)
        starts[1:] = np.cumsum(cnt)
        rank_e = np.arange(len(dl)) - starts[dl]
        bin_r = rank_e // D
        slot_in = rank_e % D
        v = R * NBINS_REG + CAPB[bin_r] + posmat[bin_r, dl]
        call = v // BINS_CALL
        vc = v % BINS_CALL
        p_ = vc % 128
        c_ = vc // 128
        sidx_pos = (8 * c_ + slot_in) * 128 + p_
        slots[call, sidx_pos] = rr.astype(np.int16)
        # scatter index blocks for this region (rank 0 of region A excluded)
        for (BR, bq), cap in zip(SCAT_BLOCKS, SCAT_CAP):
            if BR != R:
                continue
            arr = np.full(cap, TRASH, np.int64)
            dlst = dstlists[bq]
            arr[: len(dlst)] = _node_row(dlst)
            sidx_blocks.append(((BR, bq), arr))
        # sanity: region-A rank0 positions must equal dst id (r1 identity DMA)
        if R == 0:
            assert len(dstlists[0]) == PERP
            assert np.array_equal(dstlists[0], np.arange(PERP))
    # order scatter blocks as SCAT_BLOCKS
    bmap = dict(sidx_blocks)
    sidx = np.concatenate([_wrap16(bmap[key], cap // 16)
                           for key, cap in zip(SCAT_BLOCKS, SCAT_CAP)], axis=1)
    assert sidx.shape == (128, SCAT_IDX_COLS)
    return slots, sidx


def prep_inputs(x, fc1_w, fc1_b, W1, W2, bgc, fc2_w, fc2_b, src, dst):
    """Host preprocessing -> list of per-core in_maps (all device inputs)."""
    x = np.asarray(x, np.float32)
    src = np.asarray(src, np.int64)
    dst = np.asarray(dst, np.int64)

    deg_out = np.clip(np.bincount(src, minlength=N), 1, None).astype(np.float32)
    deg_in = np.clip(np.bincount(dst, minlength=N), 1, None).astype(np.float32)
    norm_s = deg_out ** -0.5
    norm_d = deg_in ** -0.5

    # global table row of each src node
    srow = PERP * (src // PER) + _node_row(src % PER)
    region = srow // REG_ROWS
    rel = srow % REG_ROWS

    core = dst // PER
    dloc = dst % PER

    betas = [float(np.log(LAMB / (l + 1) + 1.0)) for l in range(L)]
    W1b = np.stack([betas[l] * np.asarray(W1[l], np.float32) for l in range(L)])
    W2b = np.stack([betas[l] * np.asarray(W2[l], np.float32) for l in range(L)])
    # [128, 4*128]: layer l at cols 128l..128l+128  (lhsT = W[hin, hout])
    W1b_in = np.concatenate([W1b[l] for l in range(L)], axis=1).astype(np.float32)
    W2b_in = np.concatenate([W2b[l] for l in range(L)], axis=1).astype(np.float32)
    bgc_in = np.asarray(bgc, np.float32).T.copy()  # [128, 4]

    ident = np.eye(128, dtype=np.float32)

    fc1_w = np.asarray(fc1_w, np.float32)
    fc1_b = np.asarray(fc1_b, np.float32).reshape(128, 1)
    fc2_w = np.asarray(fc2_w, np.float32)
    fc2_b = np.asarray(fc2_b, np.float32).reshape(C, 1)

    in_maps = []
    for c in range(NC):
        msk = core == c
        slots, sidx = _prep_core(dloc[msk], rel[msk], region[msk])
        # gather idx wrapped per call -> [128, CALLS*512]
        gidx = np.concatenate(
            [_wrap16(slots[k], SLOTS_CALL // 16) for k in range(CALLS)], axis=1
        )
        # per-node vectors in (p, t) layout [128, TILES]
        ndloc = np.arange(PERP)
        gl = c * PER + np.minimum(ndloc, PER - 1)
        nd = ((1.0 - ALPHA) * norm_d[gl]).astype(np.float32)
        ns = norm_s[gl].astype(np.float32)
        nd[ndloc >= PER] = 0.0
        ns[ndloc >= PER] = 0.0
        normd_in = nd.reshape(TILES, 128).T.copy()  # [p, t]
        norms_in = ns.reshape(TILES, 128).T.copy()
        # xT [256, PERP]
        xT = np.zeros((IN, PERP), np.float32)
        xT[:, :PER] = x[c * PER:(c + 1) * PER].T
        in_maps.append({
            "xT": xT,
            "gidx": np.ascontiguousarray(gidx),
            "sidx": np.ascontiguousarray(sidx),
            "ident": ident,
            "fc1w": fc1_w,
            "fc1b": fc1_b,
            "W1b": W1b_in,
            "W2b": W2b_in,
            "bgcT": bgc_in,
            "fc2w": fc2_w,
            "fc2b": fc2_b,
            "normd": normd_in,
            "norms": norms_in,
        })
    return in_maps


# ---------------- device program ----------------

AL = mybir.AluOpType
AF = mybir.ActivationFunctionType

_cached = {}


PHASE_ORDER = ["fc1", "s0", "d0", "s1", "d1", "s2", "d2", "s3", "d3", "fc2"]


def build_program(upto="fc2"):
    lim = PHASE_ORDER.index(upto)
    php = lambda tag: PHASE_ORDER.index(tag) <= lim
    nc = bacc.Bacc(target_bir_lowering=False, num_devices=NC, num_swdge_queues=2)

    # external IO
    xT_d = nc.dram_tensor("xT", [IN, PERP], F32, kind="ExternalInput")
    gidx_d = nc.dram_tensor("gidx", [128, CALLS * 512], I16, kind="ExternalInput")
    sidx_d = nc.dram_tensor("sidx", [128, SCAT_IDX_COLS], I16, kind="ExternalInput")
    ident_d = nc.dram_tensor("ident", [128, 128], F32, kind="ExternalInput")
    fc1w_d = nc.dram_tensor("fc1w", [IN, H], F32, kind="ExternalInput")
    fc1b_d = nc.dram_tensor("fc1b", [H, 1], F32, kind="ExternalInput")
    W1b_d = nc.dram_tensor("W1b", [H, L * H], F32, kind="ExternalInput")
    W2b_d = nc.dram_tensor("W2b", [H, L * H], F32, kind="ExternalInput")
    bgc_d = nc.dram_tensor("bgcT", [H, L], F32, kind="ExternalInput")
    fc2w_d = nc.dram_tensor("fc2w", [H, C], F32, kind="ExternalInput")
    fc2b_d = nc.dram_tensor("fc2b", [C, 1], F32, kind="ExternalInput")
    normd_d = nc.dram_tensor("normd", [128, TILES], F32, kind="ExternalInput")
    norms_d = nc.dram_tensor("norms", [128, TILES], F32, kind="ExternalInput")
    outp_d = nc.dram_tensor("outp", [C, PERP], F32, kind="ExternalOutput")

    # internal DRAM
    tabshard = nc.dram_tensor("tabshard", [128, TILES, H], BF16)
    tabfull = nc.dram_tensor("tabfull", [NC * 128 * TILES, H], BF16, addr_space="Shared")
    agg2 = nc.dram_tensor("agg2", [PERP, H], F32)

    betas = [float(np.log(LAMB / (l + 1) + 1.0)) for l in range(L)]

    es = ExitStack()
    with es:
        ec = es.enter_context
        # SBUF
        gidx_sb = ec(nc.sbuf_tensor("gidx_sb", [128, CALLS * 512], I16))
        sidx_sb = ec(nc.sbuf_tensor("sidx_sb", [128, SCAT_IDX_COLS], I16))
        ident_sb = ec(nc.sbuf_tensor("ident_sb", [128, 128], F32))
        fc1w_sb = [ec(nc.sbuf_tensor(f"fc1w{i}", [128, H], F32)) for i in range(2)]
        fc1b_sb = ec(nc.sbuf_tensor("fc1b_sb", [H, 1], F32))
        W1b_sb = ec(nc.sbuf_tensor("W1b_sb", [H, L * H], F32))
        W2b_sb = ec(nc.sbuf_tensor("W2b_sb", [H, L * H], F32))
        bgc_sb = ec(nc.sbuf_tensor("bgc_sb", [H, L], F32))
        fc2w_sb = ec(nc.sbuf_tensor("fc2w_sb", [H, C], F32))
        fc2b_sb = ec(nc.sbuf_tensor("fc2b_sb", [C, 1], F32))
        normd_sb = ec(nc.sbuf_tensor("normd_sb", [128, TILES], F32))
        norms_sb = ec(nc.sbuf_tensor("norms_sb", [128, TILES], F32))
        stage = [ec(nc.sbuf_tensor(f"stage{i}", [128, 64, H], BF16)) for i in range(2)]
        stage2 = ec(nc.sbuf_tensor("stage2", [128, ST2_COLS, H], F32))
        f0T = ec(nc.sbuf_tensor("f0T", [128, PERP], F32))
        h_n = ec(nc.sbuf_tensor("h_n", [128, PERP], F32))
        xblk = [ec(nc.sbuf_tensor(f"xblk{i}", [128, 1024], F32)) for i in range(2)]
        aggt = [ec(nc.sbuf_tensor(f"aggt{i}", [128, 128], F32)) for i in range(2)]
        featn = [ec(nc.sbuf_tensor(f"featn{i}", [128, 128], F32)) for i in range(2)]
        featT = [ec(nc.sbuf_tensor(f"featT{i}", [128, 128], F32)) for i in range(2)]
        uT = [ec(nc.sbuf_tensor(f"uT{i}", [128, 128], F32)) for i in range(2)]
        rstT = [ec(nc.sbuf_tensor(f"rstT{i}", [128, 128], F32)) for i in range(2)]
        rstT2 = [ec(nc.sbuf_tensor(f"rstT2{i}", [128, 128], F32)) for i in range(2)]
        hn1 = [ec(nc.sbuf_tensor(f"hn1{i}", [128, 128], F32)) for i in range(2)]
        tabt = [ec(nc.sbuf_tensor(f"tabt{i}", [128, 128], BF16)) for i in range(2)]
        h4Tt = [ec(nc.sbuf_tensor(f"h4Tt{i}", [128, 128], F32)) for i in range(2)]
        outt = [ec(nc.sbuf_tensor(f"outt{i}", [C, 128], F32)) for i in range(2)]
        # PSUM
        ps_bin = [ec(nc.psum_tensor(f"psbin{i}", [128, 4, 128], F32)) for i in range(2)]
        psD = [ec(nc.psum_tensor(f"psD{i}", [128, 512], F32)) for i in range(2)]

        sem_names = ["sIN", "sX", "sG", "sMM", "sDR", "sSC", "sR1",
                     "sAG", "sTB", "sCC", "sPE", "sDV", "sAC", "sO"]
        sems = {nm: ec(nc.semaphore(nm)) for nm in sem_names}
        blk = ec(nc.Block())

        # ---- static ordinal formulas ----
        SETUP_DMAS = 13
        pe_base = lambda l: 60 + 160 * l
        dv_base = lambda l: 81 + 200 * l
        def ac_base(l):
            return 10 + sum(120 if ll < 3 else 80 for ll in range(l))
        ac_n = lambda l: 3 if l < 3 else 2
        FC2_PE = pe_base(4)
        FC2_DV = dv_base(4)
        FC2_AC = ac_base(4)

        def w(e, name, tgt):
            if tgt > 0:
                e.wait_ge(sems[name], tgt)

        # ---------------- SP: loads/stores ----------------
        @blk.sync
        def _(e):
            for dst_sb, src_dr in [
                (gidx_sb, gidx_d), (sidx_sb, sidx_d),
                (ident_sb, ident_d), (fc1b_sb, fc1b_d), (W1b_sb, W1b_d),
                (W2b_sb, W2b_d), (bgc_sb, bgc_d), (fc2w_sb, fc2w_d),
                (fc2b_sb, fc2b_d), (normd_sb, normd_d), (norms_sb, norms_d),
            ]:
                e.dma_start(out=dst_sb[:], in_=src_dr[:]).then_inc(sems["sIN"], 16)
            e.dma_start(out=fc1w_sb[0][:], in_=fc1w_d[0:128, :]).then_inc(sems["sIN"], 16)
            e.dma_start(out=fc1w_sb[1][:], in_=fc1w_d[128:256, :]).then_inc(sems["sIN"], 16)
            e.dma_start(out=xblk[0][:, 0:512], in_=xT_d[0:128, 0:512]).then_inc(sems["sX"], 16)
            # fc1 x loads (nb = node block of 512)
            for nb in range(10):
                if nb >= 2:
                    w(e, "sPE", 2 * nb - 2)
                if nb > 0:
                    e.dma_start(out=xblk[nb % 2][:, 0:512],
                                in_=xT_d[0:128, 512 * nb:512 * nb + 512]).then_inc(sems["sX"], 16)
                e.dma_start(out=xblk[nb % 2][:, 512:1024],
                            in_=xT_d[128:256, 512 * nb:512 * nb + 512]).then_inc(sems["sX"], 16)
            # fc1 table stores
            for t in range(TILES):
                w(e, "sDV", 3 + 2 * t)
                e.dma_start(out=tabshard[:, t, :], in_=tabt[t % 2][:]).then_inc(sems["sTB"], 16)
            # layers
            for l in range(L):
                if not php(f"s{l}"):
                    break
                # r1 base DMA
                w(e, "sDR", 16 * l + 5)
                w(e, "sAG", 16 * 40 * l)
                e.dma_start(out=agg2[:], in_=stage2[:, 0:TILES, :]).then_inc(sems["sR1"], 16)
                # dense loads/stores
                if not php(f"d{l}"):
                    break
                for t in range(TILES + 2):
                    if t < TILES:
                        w(e, "sSC", 16 * 9 * (l + 1))
                        if t >= 2:
                            w(e, "sDV", dv_base(l) + 5 * (t - 2) + 1)
                        e.dma_start(out=aggt[t % 2][:],
                                    in_=agg2[t::TILES, :]).then_inc(sems["sAG"], 16)
                    if l < 3 and t >= 2:
                        ts = t - 2
                        w(e, "sAC", ac_base(l) + 3 * ts + 3)
                        w(e, "sCC", l + 1)
                        e.dma_start(out=tabshard[:, ts, :],
                                    in_=tabt[ts % 2][:]).then_inc(sems["sTB"], 16)
            # fc2 stores
            for t in range(TILES):
                if not php("fc2"):
                    break
                w(e, "sAC", FC2_AC + t + 1)
                e.dma_start(out=outp_d[:, 128 * t:128 * t + 128],
                            in_=outt[t % 2][:]).then_inc(sems["sO"], 16)

        # ---------------- pool: SWDGE + collectives ----------------
        @blk.gpsimd
        def _(e):
            # AllGather 0 (fc1 table)
            w(e, "sTB", 16 * 40)
            e.collective_compute(
                "AllGather", AL.bypass, replica_groups=[list(range(NC))],
                ins=[tabshard[:].opt()], outs=[tabfull[:].opt()],
            ).then_inc(sems["sCC"])
            for l in range(L):
                if not php(f"s{l}"):
                    break
                for k in range(CALLS):
                    K = CALLS * l + k
                    w(e, "sCC", l + 1)
                    w(e, "sIN", SETUP_DMAS * 16)
                    if K >= 2:
                        w(e, "sDR", K - 1)
                    R = k // CALLS_REG
                    e.dma_gather(
                        stage[K % 2][:],
                        tabfull[R * REG_ROWS:(R + 1) * REG_ROWS, :],
                        gidx_sb[:, 512 * k:512 * k + 512],
                        SLOTS_CALL, SLOTS_CALL, H,
                        queue_num=0,
                    ).then_inc(sems["sG"], 16)
                for j in range(len(SCAT_BLOCKS)):
                    vb, cap = SCAT_VBASE[j], SCAT_CAP[j]
                    last_call = (vb + cap - 1) // BINS_CALL
                    w(e, "sDR", CALLS * l + last_call + 1)
                    w(e, "sSC", 16 * (9 * l + j))
                    if j == 0:
                        w(e, "sR1", 16 * (l + 1))
                    c0 = SCAT_COL0[j]
                    io = SCAT_IDX_OFF[j]
                    e.dma_scatter_add(
                        agg2[:],
                        stage2[:, c0:c0 + cap // 128, :],
                        sidx_sb[:, io:io + cap // 16],
                        cap, cap, H,
                        queue_num=1,
                    ).then_inc(sems["sSC"], 16)
                if l < 3 and php(f"d{l}"):
                    w(e, "sG", 16 * CALLS * (l + 1))
                    w(e, "sTB", 16 * 40 * (l + 2))
                    e.collective_compute(
                        "AllGather", AL.bypass, replica_groups=[list(range(NC))],
                        ins=[tabshard[:].opt()], outs=[tabfull[:].opt()],
                    ).then_inc(sems["sCC"])

        # ---------------- PE ----------------
        @blk.tensor
        def _(e):
            # fc1 matmuls
            for nb in range(10):
                w(e, "sX", 32 * (nb + 1))
                w(e, "sIN", SETUP_DMAS * 16)
                if nb >= 2:
                    w(e, "sAC", nb - 1)
                e.matmul(ps_bin[nb % 2][:], fc1w_sb[0][:], xblk[nb % 2][:, 0:512],
                         start=True, stop=False).then_inc(sems["sPE"], 1)
                e.matmul(ps_bin[nb % 2][:], fc1w_sb[1][:], xblk[nb % 2][:, 512:1024],
                         start=False, stop=True).then_inc(sems["sPE"], 1)
            # fc1 transposes
            for t in range(TILES):
                w(e, "sAC", t // 4 + 1)
                if t >= 2:
                    w(e, "sDV", 2 * t - 2)
                e.transpose(psD[t % 2][:, 0:128], stage2[:, t, :],
                            ident_sb[:]).then_inc(sems["sPE"], 1)
            for l in range(L):
                # dense
                if not php(f"d{l}"):
                    break
                for t in range(TILES):
                    pb = pe_base(l)
                    w(e, "sDV", dv_base(l) + 5 * t + 1)
                    if t >= 2:
                        w(e, "sDV", dv_base(l) + 5 * (t - 2) + 5)
                    e.transpose(psD[t % 2][:, 0:128], featn[t % 2][:],
                                ident_sb[:]).then_inc(sems["sPE"], 1)
                    w(e, "sDV", dv_base(l) + 5 * t + 2)
                    e.matmul(psD[t % 2][:, 128:256], W1b_sb[:, 128 * l:128 * l + 128],
                             featT[t % 2][:], start=True, stop=False).then_inc(sems["sPE"], 1)
                    e.matmul(psD[t % 2][:, 128:256], W2b_sb[:, 128 * l:128 * l + 128],
                             f0T[:, 128 * t:128 * t + 128],
                             start=False, stop=True).then_inc(sems["sPE"], 1)
                    w(e, "sAC", ac_base(l) + ac_n(l) * t + 1)
                    e.transpose(psD[t % 2][:, 256:384], rstT2[t % 2][:],
                                ident_sb[:]).then_inc(sems["sPE"], 1)
            # fc2
            for t in range(TILES):
                if not php("fc2"):
                    break
                w(e, "sAC", ac_base(3) + 2 * t + 2)
                e.transpose(psD[t % 2][:, 0:128], h_n[:, 128 * t:128 * t + 128],
                            ident_sb[:]).then_inc(sems["sPE"], 1)
                w(e, "sDV", FC2_DV + t + 1)
                if t >= 2:
                    w(e, "sAC", FC2_AC + t - 1)
                e.matmul(psD[t % 2][0:C, 384:512], fc2w_sb[:], h4Tt[t % 2][:],
                         start=True, stop=True).then_inc(sems["sPE"], 1)

        # ---------------- DVE ----------------
        @blk.vector
        def _(e):
            # fc1
            w(e, "sAC", 10)
            e.tensor_scalar(out=f0T[:], in0=stage2[:, 0:TILES, :],
                            scalar1=ALPHA, scalar2=None,
                            op0=AL.mult).then_inc(sems["sDV"], 1)
            for t in range(TILES):
                w(e, "sPE", 21 + t)
                e.tensor_copy(out=h_n[:, 128 * t:128 * t + 128],
                              in_=psD[t % 2][:, 0:128]).then_inc(sems["sDV"], 1)
                e.tensor_scalar(out=tabt[t % 2][:], in0=h_n[:, 128 * t:128 * t + 128],
                                scalar1=norms_sb[:, t:t + 1], scalar2=None,
                                op0=AL.mult).then_inc(sems["sDV"], 1)
            for l in range(L):
                # bin reduces (one per gather call)
                if not php(f"s{l}"):
                    break
                for k in range(CALLS):
                    K = CALLS * l + k
                    w(e, "sG", 16 * (K + 1))
                    if l == 0 and k == 0:
                        w(e, "sPE", 60)
                    if k == 0:
                        w(e, "sSC", 16 * 9 * l)
                        w(e, "sR1", 16 * l)
                    e.tensor_reduce(
                        out=stage2[:, 8 * k:8 * k + 8, :],
                        in_=stage[K % 2][:].rearrange("p (c i) h -> p c h i", i=8),
                        axis=mybir.AxisListType.X, op=AL.add,
                    ).then_inc(sems["sDR"], 1)
                # dense
                if not php(f"d{l}"):
                    break
                for t in range(TILES):
                    db = dv_base(l)
                    w(e, "sAG", 16 * (40 * l + t + 1))
                    if t >= 2:
                        w(e, "sPE", pe_base(l) + 4 * (t - 2) + 1)
                    e.tensor_scalar(out=featn[t % 2][:], in0=aggt[t % 2][:],
                                    scalar1=normd_sb[:, t:t + 1], scalar2=None,
                                    op0=AL.mult).then_inc(sems["sDV"], 1)
                    w(e, "sPE", pe_base(l) + 4 * t + 1)
                    if t >= 2:
                        w(e, "sPE", pe_base(l) + 4 * (t - 2) + 2)
                    e.tensor_copy(out=featT[t % 2][:],
                                  in_=psD[t % 2][:, 0:128]).then_inc(sems["sDV"], 1)
                    e.tensor_add(out=uT[t % 2][:], in0=featT[t % 2][:],
                                 in1=f0T[:, 128 * t:128 * t + 128]).then_inc(sems["sDV"], 1)
                    w(e, "sPE", pe_base(l) + 4 * t + 3)
                    if t >= 2:
                        w(e, "sAC", ac_base(l) + ac_n(l) * (t - 2) + 1)
                    e.scalar_tensor_tensor(
                        out=rstT[t % 2][:], in0=uT[t % 2][:], scalar=1.0 - betas[l],
                        in1=psD[t % 2][:, 128:256], op0=AL.mult, op1=AL.add,
                    ).then_inc(sems["sDV"], 1)
                    w(e, "sPE", pe_base(l) + 4 * t + 4)
                    tgt = ac_base(l) + ac_n(l) * (t - 2) + 3 if t >= 2 else 0
                    if l >= 1:
                        tgt = max(tgt, ac_base(l - 1) + ac_n(l - 1) * t + 2)
                    w(e, "sAC", tgt)
                    e.tensor_add(out=hn1[t % 2][:], in0=psD[t % 2][:, 256:384],
                                 in1=h_n[:, 128 * t:128 * t + 128]).then_inc(sems["sDV"], 1)
            # fc2
            for t in range(TILES):
                if not php("fc2"):
                    break
                w(e, "sPE", FC2_PE + 2 * t + 1)
                e.tensor_copy(out=h4Tt[t % 2][:],
                              in_=psD[t % 2][:, 0:128]).then_inc(sems["sDV"], 1)

        # ---------------- Act ----------------
        @blk.scalar
        def _(e):
            for nb in range(10):
                w(e, "sPE", 2 * nb + 2)
                w(e, "sIN", SETUP_DMAS * 16)
                e.activation(out=stage2[:, 4 * nb:4 * nb + 4, :], in_=ps_bin[nb % 2][:],
                             func=AF.Relu, bias=fc1b_sb[:, 0:1]).then_inc(sems["sAC"], 1)
            for l in range(L):
                if not php(f"d{l}"):
                    break
                for t in range(TILES):
                    ab = ac_base(l)
                    w(e, "sDV", dv_base(l) + 5 * t + 4)
                    if t >= 2:
                        w(e, "sPE", pe_base(l) + 4 * (t - 2) + 4)
                    e.activation(out=rstT2[t % 2][:], in_=rstT[t % 2][:],
                                 func=AF.Identity,
                                 bias=bgc_sb[:, l:l + 1]).then_inc(sems["sAC"], 1)
                    w(e, "sDV", dv_base(l) + 5 * t + 5)
                    e.activation(out=h_n[:, 128 * t:128 * t + 128], in_=hn1[t % 2][:],
                                 func=AF.Relu).then_inc(sems["sAC"], 1)
                    if l < 3:
                        if t >= 2:
                            w(e, "sTB", 16 * (40 * (l + 1) + t - 1))
                        e.activation(out=tabt[t % 2][:], in_=hn1[t % 2][:],
                                     func=AF.Relu,
                                     scale=norms_sb[:, t:t + 1]).then_inc(sems["sAC"], 1)
            # fc2
            for t in range(TILES):
                if not php("fc2"):
                    break
                w(e, "sPE", FC2_PE + 2 * t + 2)
                if t >= 2:
                    w(e, "sO", 16 * (t - 1))
                e.activation(out=outt[t % 2][:], in_=psD[t % 2][0:C, 384:512],
                             func=AF.Identity,
                             bias=fc2b_sb[:, 0:1]).then_inc(sems["sAC"], 1)

    nc.finalize()
    return nc


def kernel(x, fc1_w, fc1_b, W1, W2, bgc, fc2_w, fc2_b, src, dst):
    in_maps = prep_inputs(x, fc1_w, fc1_b, W1, W2, bgc, fc2_w, fc2_b, src, dst)
    if "nc" not in _cached:
        _cached["nc"] = build_program()
    res = run_bass_kernel_spmd(_cached["nc"], in_maps, list(range(NC)))
    out = np.empty((N, C), np.float32)
    for c in range(NC):
        ot = np.asarray(res.results[c]["outp"])  # [C, PERP]
        out[c * PER:(c + 1) * PER] = ot[:, :PER].T
    return out
